# revision 1
# baseline (speedup 1.0000x reference)
"""CapsuleLayer dynamic-routing kernel for 8 Trainium2 NeuronCores.

Problem: inputs [64, 4096, 8] f32, W [32, 4096, 16, 8] f32.
  hat[b,c,n,j] = sum_i W[c,n,j,i] * x[b,n,i]
  3 routing iterations: c = softmax_C(b); out = squash(sum_n c*hat);
  b += <out, hat>_j.

Strategy: shard the n (input-capsule) axis across the 8 cores
(N_loc = 512/core).  Everything (W shard in two bf16 layouts, x, and all
workspace) stays SBUF-resident; hat [B,C,N,D] (512MB) is never
materialized.  Per routing iteration:
  - logits beta = <outsum, hat> via a block-diagonal zero-padded matmul
    (K = 8 capsules x 16 j = 128; the block-diagonal lhsT is built each
    iteration from transposed outsum with a host-provided 0/1 mask, so
    all engine accesses stay 32-partition aligned) producing
    A[b,c,n,i] = sum_j out*W in PSUM, then an in-place multiply by x
    (mixed direct-from-PSUM 1x DVE ops and ACT-drained bf16 2x ops on
    DVE/GPSIMD, ratios tuned via DVE_DIRECT_MOD / POOL_MUL_SLOT) and a
    3-level i-tree reduction (DVE/DVE/GPSIMD).
  - softmax over capsules is local per (b,n): PE transposes the f32
    logits and ScalarE applies Exp straight from PSUM into the
    transposed e-table; the 1/Z normalization is folded into x
    (x-tilde), so couplings stay unnormalized; no max-subtraction is
    needed (logits are O(0.2)).
  - s partial = sum_n coupling*hat via per-capsule matmuls with W3
    stationary (16-column LDWEIGHTS), accumulating the 32 (i, n-tile)
    steps in PSUM; drains bounce through SBUF and SBUF->SBUF DMA to
    assemble the transposed s-layout (engines cannot touch 16-aligned
    partition bases, DMA can).
  - one [128, 256] f32 AllReduce per iteration on the transposed
    s-layout, then 4 PE transposes and squash (with a Newton-refined
    sqrt), computed identically on every core.

Since the logit update is linear in out, b_t = <sum_{t'<t} out_t', hat>,
so logits are recomputed from the running sum each iteration (no [B,C,N]
logit state).  t=0 has uniform couplings (softmax of zeros), so it is a
pure matmul with M=128-packed (capsule, j) weights.
"""

import numpy as np

B, N, I = 64, 4096, 8
C, D = 32, 16
ROUTINGS = 3
EPS = 1e-7
NCORES = 8
NL = N // NCORES          # 512 n per core
NT = NL // 128            # 4 partition tiles of n
CHUNKS = NL * I // 512    # 8 chunks of 512 along flat (n,i)


# ---------------------------------------------------------------------------
# Host-side layout prep (pure numpy, per core)
# ---------------------------------------------------------------------------

def host_prep(x, W, k):
    """Per-core input layouts for core k (n slice [k*NL, (k+1)*NL))."""
    n0 = k * NL
    Wk = np.ascontiguousarray(W[:, n0:n0 + NL])          # [C, NL, D, I]
    xk = np.ascontiguousarray(x[:, n0:n0 + NL])          # [B, NL, I]

    # W2 [128=(cp*16+j), (cg, n*8+i)]  = W[cg*8+cp, n, j, i]   (bf16)
    w2 = Wk.reshape(4, 8, NL, D, I).transpose(1, 3, 0, 2, 4).reshape(128, 4 * NL * I)
    # W3 [128=nn, (cb, nt, i, c8, j)] = W[cb*8+c8, nt*128+nn, j, i]  (bf16)
    # capsule-block-major so the t=0 phase can start on a quarter of the DMA
    w3 = Wk.reshape(4, 8, NT, 128, D, I).transpose(3, 0, 2, 5, 1, 4)            .reshape(128, NT * I * C * D)
    # xt3 [128=nn, (i, nt, b)] = x[b, nt*128+nn, i]             (bf16)
    xt3 = xk.reshape(B, NT, 128, I).transpose(2, 3, 1, 0).reshape(128, I * NT * B)
    # xr2 [64=b, (n*8+i)] = x[b, n, i]  (bf16; device duplicates rows)
    xr2 = xk.reshape(B, NL * I)

    import ml_dtypes
    bf = ml_dtypes.bfloat16
    return {
        "w2": w2.astype(bf),
        "w3": w3.astype(bf),
        "xt3": xt3.astype(bf),
        "xr2": xr2.astype(bf),
        "eyef": np.eye(128, dtype=np.float32),
        "bdmask": _bd_mask().astype(bf),
    }


_CONSTS = {}


def _prep_consts():
    if not _CONSTS:
        import ml_dtypes
        _CONSTS["eyef"] = np.eye(128, dtype=np.float32)
        _CONSTS["bdmask"] = _bd_mask().astype(ml_dtypes.bfloat16)
    return _CONSTS


def host_prep_all(x, W):
    """Vectorized host_prep for all cores at once (one cast, one transpose
    per layout).  Equivalent to [host_prep(x, W, k) for k in range(8)]."""
    import ml_dtypes
    bf = ml_dtypes.bfloat16
    Wb = np.ascontiguousarray(W, dtype=np.float32).astype(bf)   # [C, N, D, I]
    xb = np.ascontiguousarray(x, dtype=np.float32).astype(bf)   # [B, N, I]
    K = NCORES
    # w2 [k, 128=(cp,j), (cg, n, i)]
    w2 = Wb.reshape(4, 8, K, NL, D, I).transpose(2, 1, 4, 0, 3, 5)            .reshape(K, 128, 4 * NL * I)
    # w3 [k, 128=nn, (cb, nt, i, c8, j)]
    w3 = Wb.reshape(4, 8, K, NT, 128, D, I).transpose(2, 4, 0, 3, 6, 1, 5)            .reshape(K, 128, NT * I * C * D)
    # xt3 [k, 128=nn, (i, nt, b)]
    xt3 = xb.reshape(B, K, NT, 128, I).transpose(1, 3, 4, 2, 0)             .reshape(K, 128, I * NT * B)
    # xr2 [k, 64=b, (n, i)] — duplicated to both partition halves on-device
    xr2 = xb.reshape(B, K, NL * I).transpose(1, 0, 2)           # [k, 64, 4096]
    cst = _prep_consts()
    return [
        {"w2": np.ascontiguousarray(w2[k]),
         "w3": np.ascontiguousarray(w3[k]),
         "xt3": np.ascontiguousarray(xt3[k]),
         "xr2": np.ascontiguousarray(xr2[k]),
         "eyef": cst["eyef"], "bdmask": cst["bdmask"]}
        for k in range(K)
    ]


def _bd_mask():
    # mask[r, col] = 1 where ((r%32)//16) == col//64 — selects which b-half
    # of a block-diagonal lhsT tile each 16-row (one capsule's j-block) feeds.
    r = np.arange(128)[:, None]
    col = np.arange(128)[None, :]
    return (((r % 32) // 16) == (col // 64)).astype(np.float32)


# ---------------------------------------------------------------------------
# Numpy emulation of the exact device dataflow (for layout validation)
# ---------------------------------------------------------------------------

def _squash_np(s):
    # s [B, C*D] -> squash over j
    s3 = s.reshape(B, C, D)
    s2 = (s3 * s3).sum(-1)                     # [B, C]
    q = np.sqrt(s2 + EPS)
    fac = s2 / ((1.0 + s2) * q)                # [B, C]
    return (s3 * fac[:, :, None]).reshape(B, C * D)


def emulate(x, W):
    """Mirror the device program slice-for-slice in numpy (f32)."""
    per_core = [
        {k: v.astype(np.float32) for k, v in host_prep(x, W, c).items()}
        for c in range(NCORES)
    ]
    sST = [np.zeros((128, 4, 64), np.float32) for _ in range(NCORES)]
    sET = [np.zeros((128, NT, C, 64), np.float32) for _ in range(NCORES)]
    sOsum = [np.zeros((B, C * D), np.float32) for _ in range(NCORES)]
    out_t = None

    for t in range(ROUTINGS):
        for k in range(NCORES):
            io = per_core[k]
            w3cb = io["w3"].reshape(128, 4, NT, I, 8, D)
            xt3 = io["xt3"].reshape(128, I, NT, B)
            if t > 0:
                # (a) outsumT [128=(cp,j), (m, b)]; m = c-group of 8
                osumT = np.zeros((128, 4, 64), np.float32)
                for m in range(4):
                    blk = sOsum[k][:, 128 * m:128 * (m + 1)]    # [64, 128]
                    osumT[:, m, :] = blk.T
                # BD tiles [g][p]: [128, 128]
                BD = np.zeros((4, 4, 128, 128), np.float32)
                for g in range(4):
                    for p in range(4):
                        BD[g, p, 32 * p:32 * p + 16, 0:64] = osumT[32 * p:32 * p + 16, g, :]
                        BD[g, p, 32 * p + 16:32 * p + 32, 64:128] = osumT[32 * p + 16:32 * p + 32, g, :]
                # (b) A-matmuls + beta + e + ET
                w2 = io["w2"].reshape(128, 4, CHUNKS, 512)
                xr2 = np.tile(io["xr2"], (2, 1)).reshape(128, CHUNKS, 512)
                for g in range(4):
                    for p in range(4):
                        tmp = np.zeros((128, CHUNKS, 512), np.float32)
                        for ch in range(CHUNKS):
                            pA = BD[g, p].T @ w2[:, g, ch, :]   # [128=(cp2,b), 512]
                            tmp[:, ch, :] = pA * xr2[:, ch, :]
                        t8 = tmp.reshape(128, NL, I)
                        beta = t8.sum(-1)                        # [128, 512]
                        erow = np.exp(beta)
                        for nt in range(4):
                            blk = erow[:, 128 * nt:128 * (nt + 1)].T  # [128n, 128(cp2,b)]
                            c0 = g * 8 + 2 * p
                            sET[k][:, nt, c0:c0 + 2, :] = blk.reshape(128, 2, 64)
                # (c) Z, Zr, x-tilde
                Z = sET[k].transpose(0, 1, 3, 2).sum(-1)         # [128, nt, b]
                Zr = 1.0 / Z
                xtl = xt3 * Zr[:, None, :, :]                    # [128, i, nt, b]
            # (d) s-matmuls
            for cb in range(4):
                acc = np.zeros((128, 64), np.float32)
                if t == 0:
                    for i in range(I):
                        for nt in range(NT):
                            lhs = w3cb[:, cb, nt, i, :, :].reshape(128, 128)
                            acc += lhs.T @ xt3[:, i, nt, :]
                else:
                    for i in range(I):
                        for nt in range(NT):
                            et = sET[k][:, nt, cb * 8:(cb + 1) * 8, :]       # [128, 8, 64]
                            Rg = et * xtl[:, i, nt, None, :]                 # [128, 8, 64]
                            for c8 in range(8):
                                lhs = w3cb[:, cb, nt, i, c8, :]              # [128, 16]
                                acc[c8 * 16:(c8 + 1) * 16, :] += lhs.T @ Rg[:, c8, :]
                sST[k][:, cb, :] = acc * (1.0 / C if t == 0 else 1.0)
            # (e) transpose sST -> s_pre [64, (c,j)]
        # all-reduce
        s_pre = np.zeros((NCORES, B, C * D), np.float32)
        for k in range(NCORES):
            for cb in range(4):
                s_pre[k][:, cb * 128:(cb + 1) * 128] = sST[k][:, cb, :].T
        s_red = s_pre.sum(0)
        out_t = _squash_np(s_red)
        for k in range(NCORES):
            if t == 0:
                sOsum[k] = out_t.copy()
            elif t == 1:
                sOsum[k] = sOsum[k] + out_t
    return out_t.reshape(B, C, D)


# ---------------------------------------------------------------------------
# Bass device program
# ---------------------------------------------------------------------------

_CACHE = {}


DVE_DIRECT_MOD = 4
POOL_MUL_SLOT = (2,)
WP_BUFS = 2
PSA_BUFS = 3
PSO_BUFS = 4
PSE_BUFS = 2
TREE_L2_ENG = lambda nc: nc.vector


def _build_nc(sim=False, ablate=()):
    import concourse.bass as bass
    import concourse.bacc as bacc
    import concourse.mybir as mybir
    import concourse.tile as tile

    dt = mybir.dt
    f32, bf16 = dt.float32, dt.bfloat16
    ALU = mybir.AluOpType
    AF = mybir.ActivationFunctionType
    AX = mybir.AxisListType

    nc = bacc.Bacc("TRN2", target_bir_lowering=False, debug=False,
                   num_devices=NCORES)

    w2_d = nc.dram_tensor("w2", [128, 4 * NL * I], bf16, kind="ExternalInput").ap()
    w3_d = nc.dram_tensor("w3", [128, NT * I * C * D], bf16, kind="ExternalInput").ap()
    xt3_d = nc.dram_tensor("xt3", [128, I * NT * B], bf16, kind="ExternalInput").ap()
    xr2_d = nc.dram_tensor("xr2", [B, NL * I], bf16, kind="ExternalInput").ap()
    eyef_d = nc.dram_tensor("eyef", [128, 128], f32, kind="ExternalInput").ap()
    bdm_d = nc.dram_tensor("bdmask", [128, 128], bf16, kind="ExternalInput").ap()
    out_d = nc.dram_tensor("out", [B, C * D], f32, kind="ExternalOutput").ap()

    with tile.TileContext(nc) as tc:
        with (
            tc.tile_pool(name="const", bufs=1) as cp,
            tc.tile_pool(name="work", bufs=WP_BUFS) as wp,
            tc.tile_pool(name="dram", bufs=2, space="DRAM") as dp,
        ):
            sW2 = cp.tile([128, 4, CHUNKS, 512], bf16)
            sW3 = cp.tile([128, 4, NT, I, 8, D], bf16)
            sXT3 = cp.tile([128, I, NT, B], bf16)
            sXR2 = cp.tile([128, CHUNKS, 512], bf16)
            sEyeF = cp.tile([128, 128], f32)
            sBdm = cp.tile([128, 128], bf16)
            nc.sync.dma_start(sBdm[:], bdm_d[:])
            # spread the big input DMAs over distinct engine queues; W3/xT3
            # first (needed by the t=0 matmuls)
            if "nodma" in ablate:
                w3v = sW3[:].rearrange("p cb a b c d -> p (cb a b c d)")
                xt3v = sXT3[:].rearrange("p a b c -> p (a b c)")
                w2v = sW2[:].rearrange("p a b c -> p (a b c)")
                xr2v = sXR2[:].rearrange("p a b -> p (a b)")
                nc.sync.dma_start(w3v, w3_d[:].broadcast_to(w3v.shape))
                nc.scalar.dma_start(xt3v, xt3_d[:].broadcast_to(xt3v.shape))
                nc.gpsimd.dma_start(w2v, w2_d[:].broadcast_to(w2v.shape))
                nc.gpsimd.dma_start(xr2v[0:B, :], xr2_d[:])
                nc.gpsimd.dma_start(xr2v[B:128, :], xr2_d[:])
            else:
                w3v = sW3[:].rearrange("p cb a b c d -> p cb (a b c d)")
                qsz = NT * I * 8 * D
                for cbq in range(4):
                    nc.sync.dma_start(w3v[:, cbq, :],
                                      w3_d[:, cbq * qsz:(cbq + 1) * qsz])
                nc.scalar.dma_start(sXT3[:].rearrange("p a b c -> p (a b c)"), xt3_d[:])
                nc.gpsimd.dma_start(sW2[:].rearrange("p a b c -> p (a b c)"), w2_d[:])
                xrv = sXR2[:].rearrange("p a b -> p (a b)")
                nc.gpsimd.dma_start(xrv[0:B, :], xr2_d[:])
                nc.gpsimd.dma_start(xrv[B:128, :], xr2_d[:])
            nc.scalar.dma_start(sEyeF[:], eyef_d[:])

            sET = cp.tile([128, NT, C, B], bf16)
            sXt = cp.tile([128, I, NT, B], bf16)
            sZ = cp.tile([128, NT, B], f32)
            sZr = cp.tile([128, NT, B], f32)
            sST = cp.tile([128, 4, B], f32)
            sSTr = cp.tile([128, 4, B], f32)
            sS = cp.tile([B, C * D], f32)
            sOut = cp.tile([B, C * D], f32)
            sOsum = cp.tile([B, C * D], f32)
            sOsumT = cp.tile([128, 4, B], bf16)
            sBDall = cp.tile([128, 16, 128], bf16)
            sRg = cp.tile([128, I, NT, 8, B], bf16)
            # squash temps
            sq = cp.tile([B, C * D], f32)
            s2 = cp.tile([B, C], f32)
            s2e = cp.tile([B, C], f32)
            q = cp.tile([B, C], f32)
            rq = cp.tile([B, C], f32)
            q2 = cp.tile([B, C], f32)
            qs = cp.tile([B, C], f32)
            opp = cp.tile([B, C], f32)
            den = cp.tile([B, C], f32)
            rden = cp.tile([B, C], f32)
            fac = cp.tile([B, C], f32)

            nc.vector.memset(sBDall[:], 0.0)

            def squash(src, dst):
                # fully per-capsule chain: run in halves so consumers of
                # dst's leading slices (next iteration's transposes) start
                # before the second half finishes
                for h in range(2):
                    sl = slice(h * C * D // 2, (h + 1) * C * D // 2)
                    cs = slice(h * C // 2, (h + 1) * C // 2)
                    nc.vector.tensor_mul(sq[:, sl], src[:, sl], src[:, sl])
                    nc.vector.tensor_reduce(
                        s2[:, cs],
                        sq[:, sl].rearrange("b (c j) -> b c j", j=D),
                        axis=AX.X, op=ALU.add)
                    nc.vector.tensor_scalar_add(s2e[:, cs], s2[:, cs], EPS)
                    nc.scalar.sqrt(q[:, cs], s2e[:, cs])
                    nc.vector.reciprocal(rq[:, cs], q[:, cs])
                    nc.vector.tensor_mul(q2[:, cs], s2e[:, cs], rq[:, cs])
                    nc.vector.tensor_add(qs[:, cs], q[:, cs], q2[:, cs])
                    nc.vector.tensor_scalar_add(opp[:, cs], s2[:, cs], 1.0)
                    nc.vector.tensor_mul(den[:, cs], opp[:, cs], qs[:, cs])
                    nc.vector.reciprocal(rden[:, cs], den[:, cs])
                    nc.vector.tensor_mul(fac[:, cs], s2[:, cs], rden[:, cs])
                    nc.vector.tensor_scalar_mul(fac[:, cs], fac[:, cs], 2.0)
                    fb = fac[:, cs].rearrange("b (c o) -> b c o", o=1) \
                        .broadcast_to([B, C // 2, D])
                    nc.vector.tensor_mul(
                        dst[:, sl].rearrange("b (c j) -> b c j", j=D),
                        src[:, sl].rearrange("b (c j) -> b c j", j=D), fb)

            n_rout = 1 if "r1" in ablate else (2 if "r2" in ablate else ROUTINGS)
            for t in range(n_rout):
                if t > 0:
                    # (a) transpose outsum, build block-diagonal lhsT tiles
                    with tc.tile_pool(name="psO", bufs=PSO_BUFS, space="PSUM") as psO:
                        for m in range(4):
                            pT = psO.tile([128, B], f32, tag="ot")
                            nc.tensor.transpose(
                                pT[:], sOsum[:, 128 * m:128 * (m + 1)],
                                sEyeF[0:B, 0:B])
                            nc.scalar.copy(sOsumT[:, m, :], pT[:])
                    for g in range(4):
                        for p in range(4):
                            ob = sOsumT[32 * p:32 * p + 32, g, :] \
                                .rearrange("p (o b) -> p o b", o=1) \
                                .broadcast_to([32, 2, B])
                            nc.vector.tensor_mul(
                                sBDall[32 * p:32 * p + 32, g * 4 + p, :]
                                    .rearrange("p (h b) -> p h b", h=2),
                                ob,
                                sBdm[32 * p:32 * p + 32, :]
                                    .rearrange("p (h b) -> p h b", h=2))
                    # (b) A matmuls -> beta -> exp -> ET
                    with (
                        tc.tile_pool(name="psA", bufs=PSA_BUFS, space="PSUM") as psA,
                        tc.tile_pool(name="psE", bufs=PSE_BUFS, space="PSUM") as psE,
                    ):
                        for g in range(4):
                            for p in range(4):
                                tmp = wp.tile([128, CHUNKS, 512], bf16, tag="tmp")
                                tmpf = tmp
                                for c2 in range(CHUNKS // 2):
                                    pA = psA.tile([128, 1024], f32, tag="pA")
                                    for h in range(2):
                                        ch = 2 * c2 + h
                                        nc.tensor.matmul(
                                            pA[:, 512 * h:512 * (h + 1)],
                                            sBDall[:, g * 4 + p, :],
                                            sW2[:, g, ch, :],
                                            start=True, stop=True)
                                    unit = (g * 4 + p) * 4 + c2
                                    tv = tmp[:, 2 * c2:2 * c2 + 2, :] \
                                        .rearrange("p a b -> p (a b)")
                                    xv = sXR2[:, 2 * c2:2 * c2 + 2, :] \
                                        .rearrange("p a b -> p (a b)")
                                    if "bmul" in ablate and c2 > 0:
                                        pass
                                    elif unit % DVE_DIRECT_MOD == 0:
                                        # direct 1x multiply from PSUM on DVE
                                        nc.vector.tensor_mul(tv, pA[:], xv)
                                    else:
                                        # ACT drain to bf16, then 2x mul on
                                        # DVE (or GPSIMD for one unit/row)
                                        tfv = tmpf[:, 2 * c2:2 * c2 + 2, :] \
                                            .rearrange("p a b -> p (a b)")
                                        nc.scalar.copy(tfv, pA[:])
                                        meng = nc.gpsimd \
                                            if unit % 4 in POOL_MUL_SLOT \
                                            else nc.vector
                                        meng.tensor_mul(tv, tfv, xv)
                                t8 = tmp[:].rearrange("p a b -> p (a b)") \
                                           .rearrange("p (n i) -> p n i", i=I)
                                tr1 = wp.tile([128, NL, 4], bf16, tag="tr1")
                                tr2 = wp.tile([128, NL, 2], bf16, tag="tr2")
                                beta = wp.tile([128, NL, 1], f32, tag="beta")
                                if "tree" not in ablate:
                                    nc.vector.tensor_add(tr1[:], t8[:, :, 0:4], t8[:, :, 4:8])
                                    TREE_L2_ENG(nc).tensor_add(tr2[:], tr1[:, :, 0:2], tr1[:, :, 2:4])
                                    nc.gpsimd.tensor_add(beta[:], tr2[:, :, 0:1], tr2[:, :, 1:2])
                                else:
                                    nc.gpsimd.tensor_add(
                                        beta[:], t8[:, :, 0:1], t8[:, :, 1:2])
                                c0 = g * 8 + 2 * p
                                bv = beta[:].rearrange("p a b -> p (a b)")
                                pT2 = psE.tile([128, 4, 128], f32, tag="eT")
                                for nt in range(4):
                                    nc.tensor.transpose(
                                        pT2[:, nt, :], bv[:, 128 * nt:128 * (nt + 1)],
                                        sEyeF[:])
                                # one Exp over all 4 transposed chunks,
                                # PSUM -> sET directly (strided dest)
                                nc.scalar.activation(
                                    sET[:, :, c0:c0 + 2, :]
                                       .rearrange("p nt a b -> p nt (a b)"),
                                    pT2[:], AF.Exp)
                    # (c) Z = sum_c e ; x-tilde = xt3 / Z  (nt-halves pipelined)
                    for h in range(2):
                        nt0, nt1 = h * 2, h * 2 + 2
                        nc.vector.tensor_reduce(
                            sZ[:, nt0:nt1, :],
                            sET[:, nt0:nt1, :, :].rearrange("p nt c b -> p nt b c"),
                            axis=AX.X, op=ALU.add)
                        nc.vector.reciprocal(sZr[:, nt0:nt1, :], sZ[:, nt0:nt1, :])
                        zb = sZr[:, nt0:nt1, :] \
                            .rearrange("p (o nt) b -> p o nt b", o=1) \
                            .broadcast_to([128, I, 2, B])
                        nc.vector.tensor_mul(sXt[:, :, nt0:nt1, :],
                                             sXT3[:, :, nt0:nt1, :], zb)
                # (d) s matmuls
                for cb in range(4):
                    if t > 0 and "rg" not in ablate:
                        for i in range(I):
                            xb = sXt[:, i, :, :] \
                                .rearrange("p nt (o b) -> p nt o b", o=1) \
                                .broadcast_to([128, NT, 8, B])
                            nc.vector.tensor_mul(
                                sRg[:, i, :, :, :],
                                sET[:, :, cb * 8:(cb + 1) * 8, :], xb)
                    elif t > 0:
                        nc.vector.tensor_copy(
                            sRg[:].rearrange("p a b c d -> p (a b c d)"),
                            sW3[:].rearrange("p a b c d -> p (a b c d)"))
                    with tc.tile_pool(name=f"psS{t}{cb}", bufs=1, space="PSUM") as psS:
                        if t == 0:
                            pacc = psS.tile([128, B], f32, tag="s8")
                            step = 0
                            for i in range(I):
                                for nt in range(NT):
                                    lhs = sW3[:, cb, nt, i, :, :] \
                                        .rearrange("p a b -> p (a b)")
                                    nc.tensor.matmul(
                                        pacc[:], lhs, sXT3[:, i, nt, :],
                                        start=(step == 0), stop=(step == 31))
                                    step += 1
                            nc.scalar.mul(sST[:, cb, :], pacc[:], 1.0 / C)
                        else:
                            # W3 stationary (16-col LDWEIGHTS) -> out [16j, 64b]
                            paccs = [psS.tile([D, B], f32, name=f"pacc{c8}",
                                              tag=f"s{c8}")
                                     for c8 in range(8)]
                            step = 0
                            nsteps = 2 if "smm" in ablate else 32
                            for i in range(I if "smm" not in ablate else 1):
                                for nt in range(NT if "smm" not in ablate else 2):
                                    for c8 in range(8):
                                        nc.tensor.matmul(
                                            paccs[c8][:],
                                            sW3[:, cb, nt, i, c8, :],
                                            sRg[:, i, nt, c8, :],
                                            start=(step == 0),
                                            stop=(step == nsteps - 1))
                                    step += 1
                            for c8 in range(8):
                                s16 = wp.tile([D, B], f32, name=f"s16_{c8}",
                                              tag=f"s16_{c8}", bufs=2)
                                nc.scalar.copy(s16[:], paccs[c8][:])
                                nc.sync.dma_start(
                                    sST[c8 * D:(c8 + 1) * D, cb, :], s16[:])
                # all-reduce partial s across cores in sST layout, then
                # transpose to [64, (c,j)]
                if sim:
                    nc.vector.tensor_copy(sSTr[:], sST[:])
                else:
                    di = dp.tile([128, 4 * B], f32, tag="ar_in")
                    do = dp.tile([128, 4 * B], f32, tag="ar_out")
                    nc.sync.dma_start(di[:], sST[:].rearrange("p a b -> p (a b)"))
                    nc.gpsimd.collective_compute(
                        "AllReduce", mybir.AluOpType.add,
                        replica_groups=[list(range(NCORES))],
                        ins=[di[:].opt()], outs=[do[:].opt()])
                    nc.sync.dma_start(sSTr[:].rearrange("p a b -> p (a b)"), do[:])
                with tc.tile_pool(name=f"psT{t}", bufs=2, space="PSUM") as psT:
                    for cb in range(4):
                        pT3 = psT.tile([B, 128], f32, tag="sT")
                        nc.tensor.transpose(pT3[:], sSTr[:, cb, :], sEyeF[:])
                        nc.scalar.copy(sS[:, cb * 128:(cb + 1) * 128], pT3[:])
                squash(sS, sOut)
                if t == n_rout - 1:
                    nc.sync.dma_start(out_d[:], sOut[:])
                elif t == 0:
                    for h in range(2):
                        sl = slice(h * C * D // 2, (h + 1) * C * D // 2)
                        nc.vector.tensor_copy(sOsum[:, sl], sOut[:, sl])
                else:
                    for h in range(2):
                        sl = slice(h * C * D // 2, (h + 1) * C * D // 2)
                        nc.vector.tensor_add(sOsum[:, sl], sOsum[:, sl],
                                             sOut[:, sl])
    nc.compile()
    return nc


def get_nc(sim=False, ablate=()):
    key = ("nc_sim" if sim else "nc") + "_".join(ablate)
    if key not in _CACHE:
        _CACHE[key] = _build_nc(sim=sim, ablate=ablate)
    return _CACHE[key]


def kernel(inputs, W):
    inputs = np.asarray(inputs, dtype=np.float32)
    W = np.asarray(W, dtype=np.float32)
    nc = get_nc()
    in_maps = host_prep_all(inputs, W)
    from concourse import bass_utils
    res = bass_utils.run_bass_kernel_spmd(
        nc, in_maps, core_ids=list(range(NCORES)))
    return res.results[0]["out"].reshape(B, C, D).astype(np.float32)

